# revision 8
# baseline (speedup 1.0000x reference)
"""2D DCT-II (ortho) over the last two axes of x[8, 32, 512, 512] (f32),
data-parallel across 8 NeuronCores (one batch element per core).

Four-quadrant even/odd decomposition: with A = D[0::2, :256],
B = D[1::2, :256], row-fold R+/- = X[i] +/- X[511-i] and col-fold
Q{s,t} = R_s[:, j] +/- R_s[:, 511-j], the output splits into
  Y[2a+s, 2b+t] = (S_s Q_{s,t} S_t^T)[a, b],  S_0 = A, S_1 = B,
so both matmul stages contract over 256 instead of 512 (134M MACs/image
vs 201M for the col-fold-only version).

All device compute is bf16 (f32 PSUM accumulate); the host converts
inputs to bf16 and upcasts the bf16 result (tolerance is 2e-2).
Host-side layout prep keeps every DMA descriptor 2-4KB and every
on-chip op plainly strided:
  - bottom half of X uploaded row-reversed (row fold needs partition
    alignment of row i with row 511-i);
  - columns uploaded as [0..255, 511..256] so the col fold is a plain
    first-half/second-half add;
  - output stored as [p, ab, s, t, b] (row u = 256*ab + 2p + s,
    col v = 2b + t) and de-interleaved on the host.
"""
import numpy as np
import ml_dtypes

import concourse.bass as bass
import concourse.mybir as mybir
import concourse.tile as tile
from concourse.bass_utils import run_bass_kernel_spmd

P = 128
N = 512
H = N // 2          # 256
NIMG = 32
NCORES = 8

_MAX_WAITS = 1


def _split_excess_waits(nc):
    """walrus CoreV3 codegen rejects instructions carrying several sem
    waits; hoist excess waits onto preceding same-engine NoOps."""
    for f in nc.m.functions:
        for bb in f.blocks:
            insts = bb.instructions
            i = 0
            while i < len(insts):
                inst = insts[i]
                si = inst.sync_info
                if si is not None and si.on_wait and len(si.on_wait) > _MAX_WAITS:
                    waits = list(si.on_wait)
                    keep = waits[-_MAX_WAITS:]
                    hoist = waits[:-_MAX_WAITS]
                    nops = []
                    for w in hoist:
                        nop = mybir.InstNoOp(
                            name=nc.get_next_instruction_name(), ins=[], outs=[])
                        nop.engine = inst.engine
                        nop.sync_info = mybir.SyncInfo(on_wait=[w], on_update=[])
                        nops.append(nop)
                    si.on_wait = keep
                    for off, nop in enumerate(nops):
                        insts.insert(i + off, nop)
                    i += len(nops)
                i += 1


def _dct_mats():
    k = np.arange(N)[:, None]
    j = np.arange(N)[None, :]
    D = np.cos(np.pi * (2 * j + 1) * k / (2.0 * N))
    D *= np.sqrt(2.0 / N)
    D[0] *= 1.0 / np.sqrt(2.0)
    A = D[0::2, :H]                              # [a, i]
    B = D[1::2, :H]
    bf = ml_dtypes.bfloat16
    at1 = np.ascontiguousarray(A.T.reshape(P, 2, H)).astype(bf)
    bt1 = np.ascontiguousarray(B.T.reshape(P, 2, H)).astype(bf)
    at2 = np.ascontiguousarray(A.T.reshape(2, P, H).transpose(1, 0, 2)).astype(bf)
    bt2 = np.ascontiguousarray(B.T.reshape(2, P, H).transpose(1, 0, 2)).astype(bf)
    return at1, bt1, at2, bt2


def _build(split_waits=True):
    nc = bass.Bass()
    f32 = mybir.dt.float32
    bf16 = mybir.dt.bfloat16
    xt_d = nc.dram_tensor("xt", [NIMG, P, 2, N], bf16, kind="ExternalInput")
    xb_d = nc.dram_tensor("xb", [NIMG, P, 2, N], bf16, kind="ExternalInput")
    at1_d = nc.dram_tensor("at1", [P, 2, H], bf16, kind="ExternalInput")
    bt1_d = nc.dram_tensor("bt1", [P, 2, H], bf16, kind="ExternalInput")
    at2_d = nc.dram_tensor("at2", [P, 2, H], bf16, kind="ExternalInput")
    bt2_d = nc.dram_tensor("bt2", [P, 2, H], bf16, kind="ExternalInput")
    y_d = nc.dram_tensor("y", [NIMG, P, 2, 2, 2, H], bf16, kind="ExternalOutput")

    with tile.TileContext(nc) as tc:
        with (
            tc.tile_pool(name="const", bufs=1) as cpool,
            tc.tile_pool(name="xp", bufs=3) as xp,
            tc.tile_pool(name="rp", bufs=2) as rp,
            tc.tile_pool(name="qp", bufs=2) as qp,
            tc.tile_pool(name="zp", bufs=2) as zp,
            tc.tile_pool(name="yp", bufs=3) as yp,
            tc.tile_pool(name="ps1", bufs=3, space="PSUM") as ps1,
            tc.tile_pool(name="ps2", bufs=3, space="PSUM") as ps2,
        ):
            s1rhs = []
            for nm, d in (("at1", at1_d), ("bt1", bt1_d)):
                t = cpool.tile([P, 2, H], bf16, tag=nm)
                nc.sync.dma_start(t[:], d[:])
                s1rhs.append(t)
            s2rhs = []
            for nm, d in (("at2", at2_d), ("bt2", bt2_d)):
                t = cpool.tile([P, 2, H], bf16, tag=nm)
                nc.sync.dma_start(t[:], d[:])
                s2rhs.append(t)

            for img in range(NIMG):
                tt = xp.tile([P, 2, N], bf16, tag="t")
                bb = xp.tile([P, 2, N], bf16, tag="b")
                nc.sync.dma_start(tt[:], xt_d[img])
                nc.sync.dma_start(bb[:], xb_d[img])

                # row fold (partition-aligned thanks to reversed upload).
                # gpsimd (Pool) is SBUF-only, so it gets fold work while
                # vector/scalar handle the PSUM reads below.
                r = [rp.tile([P, 2, N], bf16, tag=f"r{s}", name=f"r{s}")
                     for s in range(2)]
                nc.gpsimd.tensor_add(r[0][:], tt[:], bb[:])
                nc.gpsimd.tensor_sub(r[1][:], tt[:], bb[:])

                # col fold (plain halves thanks to col-permuted upload)
                q = [[qp.tile([P, 2, H], bf16, tag=f"q{s}{t}", name=f"q{s}{t}")
                      for t in range(2)] for s in range(2)]
                for s in range(2):
                    nc.vector.tensor_add(
                        q[s][0][:], r[s][:, :, 0:H], r[s][:, :, H:N])
                    nc.gpsimd.tensor_sub(
                        q[s][1][:], r[s][:, :, 0:H], r[s][:, :, H:N])

                # stage 1: Z_st[j, a] = sum_i Q_st[i, j] * S_s[a, i]
                z = [[zp.tile([P, 2, H], bf16, tag=f"z{s}{t}", name=f"z{s}{t}")
                      for t in range(2)] for s in range(2)]
                for s in range(2):
                    for t in range(2):
                        pz = ps1.tile([P, 2, H], f32, tag="pz")
                        for jb in range(2):
                            for ro in range(2):
                                nc.tensor.matmul(
                                    pz[:, jb, :],
                                    q[s][t][:, ro, jb * P:(jb + 1) * P],
                                    s1rhs[s][:, ro, :],
                                    start=(ro == 0),
                                    stop=(ro == 1),
                                )
                        eng = nc.scalar.copy if t == 0 else nc.vector.tensor_copy
                        eng(z[s][t][:], pz[:])

                # stage 2: Y_st[a, b] = sum_j Z_st[j, a] * S_t[b, j]
                ysb = yp.tile([P, 2, 2, 2, H], bf16)
                for s in range(2):
                    for ab in range(2):
                        py = ps2.tile([P, 2, H], f32, tag="py")
                        for t in range(2):
                            for jb in range(2):
                                nc.tensor.matmul(
                                    py[:, t, :],
                                    z[s][t][:, jb, ab * P:(ab + 1) * P],
                                    s2rhs[t][:, jb, :],
                                    start=(jb == 0),
                                    stop=(jb == 1),
                                )
                        eng = nc.scalar.copy if ab == 0 else nc.vector.tensor_copy
                        eng(ysb[:, ab, s], py[:])
                nc.sync.dma_start(y_d[img], ysb[:])

    if split_waits:
        _split_excess_waits(nc)
    return nc


_CACHE = {}


def _get_nc():
    if "nc" not in _CACHE:
        _CACHE["nc"] = _build()
    return _CACHE["nc"]


def _host_prep(xc):
    """xc [NIMG, 512, 512] f32 (one core) -> xt, xb bf16 [NIMG, P, 2, N]."""
    bf = ml_dtypes.bfloat16
    top = xc[:, :H, :]
    bot = xc[:, :H - 1:-1, :]        # rows 511..256: index i <-> row 511-i
    out = []
    for h in (top, bot):
        hp = np.concatenate([h[..., :H], h[..., :H - 1:-1]], axis=-1)
        out.append(np.ascontiguousarray(hp.reshape(NIMG, P, 2, N)).astype(bf))
    return out


def _in_maps(x):
    at1, bt1, at2, bt2 = _dct_mats()
    maps = []
    for i in range(NCORES):
        xt, xb = _host_prep(x[i])
        maps.append({"xt": xt, "xb": xb,
                     "at1": at1, "bt1": bt1, "at2": at2, "bt2": bt2})
    return maps


def _host_post(y_hw):
    """y_hw [NIMG, P, 2, 2, 2, H] bf16 -> Y [NIMG, 512, 512] f32."""
    Y = np.empty((NIMG, N, N), dtype=np.float32)
    view = Y.reshape(NIMG, 2, P, 2, H, 2)           # [img, ab, p, s, b, t]
    view[...] = y_hw.astype(np.float32).transpose(0, 2, 1, 3, 5, 4)
    return Y


def kernel(x):
    x = np.ascontiguousarray(np.asarray(x, dtype=np.float32))
    assert x.shape == (NCORES, NIMG, N, N), x.shape
    nc = _get_nc()
    res = run_bass_kernel_spmd(nc, _in_maps(x), core_ids=list(range(NCORES)))
    out = np.stack([_host_post(res.results[i]["y"]) for i in range(NCORES)],
                   axis=0)
    return out.astype(np.float32)


# revision 10
# speedup vs baseline: 1.5696x; 1.5696x over previous
"""2D DCT-II (ortho) over the last two axes of x[8, 32, 512, 512] (f32),
data-parallel across 8 NeuronCores (one batch element per core).

Four-quadrant even/odd decomposition: with A = D[0::2, :256],
B = D[1::2, :256], row-fold R+/- = X[i] +/- X[511-i] and col-fold
Q{s,t} = R_s[:, j] +/- R_s[:, 511-j], the output splits into
  Y[2a+s, 2b+t] = (S_s Q_{s,t} S_t^T)[a, b],  S_0 = A, S_1 = B,
so both matmul stages contract over 256 instead of 512 (134M MACs/image
vs 201M for the col-fold-only version).

All device compute is bf16 (f32 PSUM accumulate); the host converts
inputs to bf16 and upcasts the bf16 result (tolerance is 2e-2).
Host-side layout prep keeps every DMA descriptor 2-4KB and every
on-chip op plainly strided:
  - bottom half of X uploaded row-reversed (row fold needs partition
    alignment of row i with row 511-i);
  - columns uploaded as [0..255, 511..256] so the col fold is a plain
    first-half/second-half add;
  - output stored as [p, ab, s, t, b] (row u = 256*ab + 2p + s,
    col v = 2b + t) and de-interleaved on the host.
"""
import numpy as np
import ml_dtypes

import concourse.bass as bass
import concourse.mybir as mybir
import concourse.tile as tile
from concourse.bass_utils import run_bass_kernel_spmd

P = 128
N = 512
H = N // 2          # 256
NIMG = 32
NCORES = 8

_MAX_WAITS = 1


def _split_excess_waits(nc):
    """walrus CoreV3 codegen rejects instructions carrying several sem
    waits; hoist excess waits onto preceding same-engine NoOps."""
    for f in nc.m.functions:
        for bb in f.blocks:
            insts = bb.instructions
            i = 0
            while i < len(insts):
                inst = insts[i]
                si = inst.sync_info
                if si is not None and si.on_wait and len(si.on_wait) > _MAX_WAITS:
                    waits = list(si.on_wait)
                    keep = waits[-_MAX_WAITS:]
                    hoist = waits[:-_MAX_WAITS]
                    nops = []
                    for w in hoist:
                        nop = mybir.InstNoOp(
                            name=nc.get_next_instruction_name(), ins=[], outs=[])
                        nop.engine = inst.engine
                        nop.sync_info = mybir.SyncInfo(on_wait=[w], on_update=[])
                        nops.append(nop)
                    si.on_wait = keep
                    for off, nop in enumerate(nops):
                        insts.insert(i + off, nop)
                    i += len(nops)
                i += 1


def _dct_mats():
    k = np.arange(N)[:, None]
    j = np.arange(N)[None, :]
    D = np.cos(np.pi * (2 * j + 1) * k / (2.0 * N))
    D *= np.sqrt(2.0 / N)
    D[0] *= 1.0 / np.sqrt(2.0)
    A = D[0::2, :H]                              # [a, i]
    B = D[1::2, :H]
    bf = ml_dtypes.bfloat16
    at1 = np.ascontiguousarray(A.T.reshape(P, 2, H)).astype(bf)
    bt1 = np.ascontiguousarray(B.T.reshape(P, 2, H)).astype(bf)
    at2 = np.ascontiguousarray(A.T.reshape(2, P, H).transpose(1, 0, 2)).astype(bf)
    bt2 = np.ascontiguousarray(B.T.reshape(2, P, H).transpose(1, 0, 2)).astype(bf)
    return at1, bt1, at2, bt2


def _build(split_waits=True):
    nc = bass.Bass()
    f32 = mybir.dt.float32
    bf16 = mybir.dt.bfloat16
    xt_d = nc.dram_tensor("xt", [NIMG, P, 2, N], bf16, kind="ExternalInput")
    xb_d = nc.dram_tensor("xb", [NIMG, P, 2, N], bf16, kind="ExternalInput")
    at1_d = nc.dram_tensor("at1", [P, 2, H], bf16, kind="ExternalInput")
    bt1_d = nc.dram_tensor("bt1", [P, 2, H], bf16, kind="ExternalInput")
    at2_d = nc.dram_tensor("at2", [P, 2, H], bf16, kind="ExternalInput")
    bt2_d = nc.dram_tensor("bt2", [P, 2, H], bf16, kind="ExternalInput")
    y_d = nc.dram_tensor("y", [NIMG, P, 2, 2, 2, H], bf16, kind="ExternalOutput")

    with tile.TileContext(nc) as tc:
        with (
            tc.tile_pool(name="const", bufs=1) as cpool,
            tc.tile_pool(name="xp", bufs=3) as xp,
            tc.tile_pool(name="rp", bufs=2) as rp,
            tc.tile_pool(name="qp", bufs=2) as qp,
            tc.tile_pool(name="zp", bufs=2) as zp,
            tc.tile_pool(name="yp", bufs=3) as yp,
            tc.tile_pool(name="ps1", bufs=2, space="PSUM") as ps1,
            tc.tile_pool(name="ps2", bufs=2, space="PSUM") as ps2,
        ):
            s1rhs = []
            for nm, d in (("at1", at1_d), ("bt1", bt1_d)):
                t = cpool.tile([P, 2, H], bf16, tag=nm)
                nc.sync.dma_start(t[:], d[:])
                s1rhs.append(t)
            s2rhs = []
            for nm, d in (("at2", at2_d), ("bt2", bt2_d)):
                t = cpool.tile([P, 2, H], bf16, tag=nm)
                nc.sync.dma_start(t[:], d[:])
                s2rhs.append(t)

            for img in range(NIMG):
                tt = xp.tile([P, 2, N], bf16, tag="t")
                bb = xp.tile([P, 2, N], bf16, tag="b")
                nc.sync.dma_start(tt[:], xt_d[img])
                nc.sync.dma_start(bb[:], xb_d[img])

                # Folds all on DVE: all-bf16 packed operands hit the 2x_1p
                # fast path.  GpSimd/Pool is unused on purpose — it shares
                # SBUF ports with DVE and halves both engines' throughput.
                # row fold (partition-aligned thanks to reversed upload)
                r = [rp.tile([P, 2, N], bf16, tag=f"r{s}", name=f"r{s}")
                     for s in range(2)]
                nc.vector.tensor_add(r[0][:], tt[:], bb[:])
                nc.vector.tensor_sub(r[1][:], tt[:], bb[:])

                # col fold (plain halves thanks to col-permuted upload)
                q = [[qp.tile([P, 2, H], bf16, tag=f"q{s}{t}", name=f"q{s}{t}")
                      for t in range(2)] for s in range(2)]
                for s in range(2):
                    nc.vector.tensor_add(
                        q[s][0][:], r[s][:, :, 0:H], r[s][:, :, H:N])
                    nc.vector.tensor_sub(
                        q[s][1][:], r[s][:, :, 0:H], r[s][:, :, H:N])

                # stage 1: Z_st[j, a] = sum_i Q_st[i, j] * S_s[a, i]
                # Both t-quadrants of one s share a 2-bank PSUM tile so the
                # PSUM->SBUF copy is one big op (amortizes access latency).
                z = [zp.tile([P, 2, 2, H], bf16, tag=f"z{s}", name=f"z{s}")
                     for s in range(2)]
                for s in range(2):
                    pz = ps1.tile([P, 2, 2, H], f32, tag="pz")
                    for t in range(2):
                        for jb in range(2):
                            for ro in range(2):
                                nc.tensor.matmul(
                                    pz[:, t, jb, :],
                                    q[s][t][:, ro, jb * P:(jb + 1) * P],
                                    s1rhs[s][:, ro, :],
                                    start=(ro == 0),
                                    stop=(ro == 1),
                                )
                    eng = nc.vector.tensor_copy if s == 0 else nc.scalar.copy
                    eng(z[s][:], pz[:])

                # stage 2: Y_st[a, b] = sum_j Z_st[j, a] * S_t[b, j]
                ysb = yp.tile([P, 2, 2, 2, H], bf16)
                for s in range(2):
                    py = ps2.tile([P, 2, 2, H], f32, tag="py")
                    for ab in range(2):
                        for t in range(2):
                            for jb in range(2):
                                nc.tensor.matmul(
                                    py[:, ab, t, :],
                                    z[s][:, t, jb, ab * P:(ab + 1) * P],
                                    s2rhs[t][:, jb, :],
                                    start=(jb == 0),
                                    stop=(jb == 1),
                                )
                    nc.scalar.copy(ysb[:, :, s], py[:])
                nc.sync.dma_start(y_d[img], ysb[:])

    if split_waits:
        _split_excess_waits(nc)
    return nc


_CACHE = {}


def _get_nc():
    if "nc" not in _CACHE:
        _CACHE["nc"] = _build()
    return _CACHE["nc"]


def _host_prep(xc):
    """xc [NIMG, 512, 512] f32 (one core) -> xt, xb bf16 [NIMG, P, 2, N]."""
    bf = ml_dtypes.bfloat16
    top = xc[:, :H, :]
    bot = xc[:, :H - 1:-1, :]        # rows 511..256: index i <-> row 511-i
    out = []
    for h in (top, bot):
        hp = np.concatenate([h[..., :H], h[..., :H - 1:-1]], axis=-1)
        out.append(np.ascontiguousarray(hp.reshape(NIMG, P, 2, N)).astype(bf))
    return out


def _in_maps(x):
    at1, bt1, at2, bt2 = _dct_mats()
    maps = []
    for i in range(NCORES):
        xt, xb = _host_prep(x[i])
        maps.append({"xt": xt, "xb": xb,
                     "at1": at1, "bt1": bt1, "at2": at2, "bt2": bt2})
    return maps


def _host_post(y_hw):
    """y_hw [NIMG, P, 2, 2, 2, H] bf16 -> Y [NIMG, 512, 512] f32."""
    Y = np.empty((NIMG, N, N), dtype=np.float32)
    view = Y.reshape(NIMG, 2, P, 2, H, 2)           # [img, ab, p, s, b, t]
    view[...] = y_hw.astype(np.float32).transpose(0, 2, 1, 3, 5, 4)
    return Y


def kernel(x):
    x = np.ascontiguousarray(np.asarray(x, dtype=np.float32))
    assert x.shape == (NCORES, NIMG, N, N), x.shape
    nc = _get_nc()
    res = run_bass_kernel_spmd(nc, _in_maps(x), core_ids=list(range(NCORES)))
    out = np.stack([_host_post(res.results[i]["y"]) for i in range(NCORES)],
                   axis=0)
    return out.astype(np.float32)
